# revision 1
# baseline (speedup 1.0000x reference)
"""GRU cell (nn.GRUCell) on 8 Trainium2 NeuronCores.

Strategy: data-parallel over the batch dim (16384 -> 2048 rows/core).
The 6 weight matrices are concatenated host-side into Wi=[IN,3H],
Wh=[H,3H] and replicated to every core; x_t/h_t are pre-transposed on
the host so each core can use batch-column slices of x^T/h^T directly
as the matmul stationary operand (lhsT, contraction dim on partitions)
without any on-device transpose.

Matmul dtype: float32r (fp32 bits, PE "replicated" mode) keeps near-fp32
accuracy; bf16 halves DMA traffic but amplifies error through the
r * gh_n product. Selected by `MM_DTYPE`.

Per core, per 128-row batch tile:
  r/z gates accumulate x- and h-side contributions in one PSUM group
  (8 matmuls each); the n gate needs gi_n and gh_n separately (the
  reset gate scales only the hidden contribution), 4 matmuls each.
  Gates are evaluated with ScalarE sigmoid/tanh straight out of PSUM,
  the blend (1-z)*n + z*h runs on VectorE in fp32, and the result is
  DMA'd out in natural [batch, H] layout.
"""

import numpy as np
import ml_dtypes

import concourse.mybir as mybir
from concourse import bacc
import concourse.tile as tile
from concourse.bass_utils import run_bass_kernel_spmd

N_CORES = 8
B, IN, H = 16384, 512, 512
BL = B // N_CORES          # batch rows per core
P = 128                    # SBUF partitions
MT = BL // P               # batch tiles per core
KC = IN // P               # contraction chunks per GEMM side
H3 = 3 * H
F32 = mybir.dt.float32
F32R = mybir.dt.float32r
BF16 = mybir.dt.bfloat16
SIG = mybir.ActivationFunctionType.Sigmoid
TANH = mybir.ActivationFunctionType.Tanh

MM_DTYPE = "f32r"          # "f32r" | "bf16"
HN_BF16 = "direct"         # False: f32 load; "dma": cast-on-DMA; "direct": bf16 tile into DVE


def build_program(
    use_bias=False,
    loop_n=1,
    enable_asserts=False,
    mm_dtype=None,
    unroll=1,
    resident_outside=False,
):
    mm_dtype = mm_dtype or MM_DTYPE
    in_dt = BF16 if mm_dtype == "bf16" else F32R

    def mm_ap(ap):
        return ap

    nc = bacc.Bacc(
        "TRN2",
        target_bir_lowering=False,
        debug=False,
        enable_asserts=enable_asserts,
        num_devices=N_CORES,
    )
    xT = nc.dram_tensor("xT", [IN, BL], in_dt, kind="ExternalInput").ap()
    hT = nc.dram_tensor("hT", [H, BL], in_dt, kind="ExternalInput").ap()
    hN = nc.dram_tensor(
        "hN", [BL, H], BF16 if HN_BF16 else F32, kind="ExternalInput"
    ).ap()
    wi = nc.dram_tensor("wi", [IN, H3], in_dt, kind="ExternalInput").ap()
    wh = nc.dram_tensor("wh", [H, H3], in_dt, kind="ExternalInput").ap()
    bias = (
        nc.dram_tensor("bias", [P, H3], F32, kind="ExternalInput").ap()
        if use_bias
        else None
    )
    out = nc.dram_tensor("out", [BL, H], F32, kind="ExternalOutput").ap()

    xT_c = xT.rearrange("(k p) b -> k p b", p=P)    # [KC, 128, BL]
    hT_c = hT.rearrange("(k p) b -> k p b", p=P)
    wi_c = wi.rearrange("(k p) n -> k p n", p=P)    # [KC, 128, 3H]
    wh_c = wh.rearrange("(k p) n -> k p n", p=P)
    hN_m = hN.rearrange("(m p) n -> m p n", p=P)    # [MT, 128, H]
    out_m = out.rearrange("(m p) n -> m p n", p=P)

    with tile.TileContext(nc) as tc:
        with (
            tc.tile_pool(name="resident", bufs=1) as rpool,
            tc.tile_pool(name="stream", bufs=3) as spool,
            tc.tile_pool(name="psum", bufs=2, space="PSUM") as ppool,
        ):

            def load_resident():
                wi_sb, wh_sb = [], []
                for k in range(KC):
                    t = rpool.tile([P, H3], in_dt, tag=f"wi{k}")
                    nc.sync.dma_start(out=t, in_=wi_c[k])
                    wi_sb.append(t)
                for k in range(KC):
                    t = rpool.tile([P, H3], in_dt, tag=f"wh{k}")
                    nc.sync.dma_start(out=t, in_=wh_c[k])
                    wh_sb.append(t)
                b_sb = None
                if use_bias:
                    b_sb = rpool.tile([P, H3], F32, tag="bias")
                    nc.sync.dma_start(out=b_sb, in_=bias)
                return wi_sb, wh_sb, b_sb

            GS = 4               # m-tiles per activation slice group
            W_SL = GS * P        # slice width in batch columns

            def body(res):
                wi_sb, wh_sb, b_sb = res
                for m in range(MT):
                    mi = m % GS
                    if mi == 0:
                        g = m // GS
                        gsl = slice(g * W_SL, (g + 1) * W_SL)
                        xs, hs = [], []
                        for k in range(KC):
                            t = spool.tile([P, W_SL], in_dt, tag=f"xs{k}")
                            nc.sync.dma_start(out=t, in_=xT_c[k][:, gsl])
                            xs.append(t)
                        for k in range(KC):
                            t = spool.tile([P, W_SL], in_dt, tag=f"hs{k}")
                            nc.sync.dma_start(out=t, in_=hT_c[k][:, gsl])
                            hs.append(t)

                    if HN_BF16 == "direct":
                        h_sb = spool.tile([P, H], BF16, tag="h")
                        nc.sync.dma_start(out=h_sb, in_=hN_m[m])
                    elif HN_BF16 == "dma":
                        h_sb = spool.tile([P, H], F32, tag="h")
                        # bf16 -> f32 cast during DMA (SWDGE)
                        nc.gpsimd.dma_start(out=h_sb, in_=hN_m[m])
                    else:
                        h_sb = spool.tile([P, H], F32, tag="h")
                        nc.sync.dma_start(out=h_sb, in_=hN_m[m])

                    r_ps = ppool.tile([P, H], F32, tag="r")
                    z_ps = ppool.tile([P, H], F32, tag="z")
                    gin_ps = ppool.tile([P, H], F32, tag="gin")
                    ghn_ps = ppool.tile([P, H], F32, tag="ghn")

                    ms = slice(mi * P, (mi + 1) * P)
                    for k in range(KC):
                        lx = mm_ap(xs[k][:, ms])
                        w = wi_sb[k]
                        nc.tensor.matmul(
                            r_ps, lx, mm_ap(w[:, 0:H]), start=(k == 0), stop=False
                        )
                        nc.tensor.matmul(
                            z_ps, lx, mm_ap(w[:, H : 2 * H]), start=(k == 0), stop=False
                        )
                        nc.tensor.matmul(
                            gin_ps,
                            lx,
                            mm_ap(w[:, 2 * H : 3 * H]),
                            start=(k == 0),
                            stop=(k == KC - 1),
                        )
                    for k in range(KC):
                        lh = mm_ap(hs[k][:, ms])
                        w = wh_sb[k]
                        nc.tensor.matmul(
                            r_ps, lh, mm_ap(w[:, 0:H]), start=False, stop=(k == KC - 1)
                        )
                        nc.tensor.matmul(
                            z_ps,
                            lh,
                            mm_ap(w[:, H : 2 * H]),
                            start=False,
                            stop=(k == KC - 1),
                        )
                        nc.tensor.matmul(
                            ghn_ps,
                            lh,
                            mm_ap(w[:, 2 * H : 3 * H]),
                            start=(k == 0),
                            stop=(k == KC - 1),
                        )

                    if use_bias:
                        nc.vector.tensor_add(r_ps, r_ps, b_sb[:, 0:H])
                        nc.vector.tensor_add(z_ps, z_ps, b_sb[:, H : 2 * H])
                        nc.vector.tensor_add(gin_ps, gin_ps, b_sb[:, 2 * H : 3 * H])

                    r_sb = spool.tile([P, H], F32, tag="r_sb")
                    nc.scalar.activation(r_sb, r_ps, SIG)
                    z_sb = spool.tile([P, H], F32, tag="z_sb")
                    nc.scalar.activation(z_sb, z_ps, SIG)

                    t_sb = spool.tile([P, H], F32, tag="t_sb")
                    nc.vector.tensor_mul(t_sb, r_sb, ghn_ps)
                    np_sb = spool.tile([P, H], F32, tag="np_sb")
                    nc.vector.tensor_add(np_sb, t_sb, gin_ps)
                    n_sb = spool.tile([P, H], F32, tag="n_sb")
                    nc.scalar.activation(n_sb, np_sb, TANH)

                    # out = n + z * (h - n)  ==  (1-z)*n + z*h
                    d_sb = spool.tile([P, H], F32, tag="d_sb")
                    nc.vector.tensor_sub(d_sb, h_sb, n_sb)
                    e_sb = spool.tile([P, H], F32, tag="e_sb")
                    nc.vector.tensor_mul(e_sb, z_sb, d_sb)
                    o_sb = spool.tile([P, H], F32, tag="o_sb")
                    nc.vector.tensor_add(o_sb, n_sb, e_sb)
                    nc.sync.dma_start(out=out_m[m], in_=o_sb)

            if loop_n == 1:
                res = load_resident()
                for _ in range(unroll):
                    body(res)
            elif resident_outside:
                res = load_resident()
                with tc.For_i(0, loop_n, 1):
                    for _ in range(unroll):
                        body(res)
            else:
                with tc.For_i(0, loop_n, 1):
                    for _ in range(unroll):
                        res = load_resident()
                        body(res)

    nc.compile()
    return nc


def make_in_maps(
    x_t, h_t, W_ir, W_hr, b_r, W_iz, W_hz, b_z, W_in, W_hn, b_n, mm_dtype=None
):
    mm_dtype = mm_dtype or MM_DTYPE
    np_dt = ml_dtypes.bfloat16 if mm_dtype == "bf16" else np.float32
    x_t = np.asarray(x_t, dtype=np.float32)
    h_t = np.asarray(h_t, dtype=np.float32)
    Wi = np.concatenate(
        [np.asarray(W_ir), np.asarray(W_iz), np.asarray(W_in)], axis=1
    ).astype(np_dt)
    Wh = np.concatenate(
        [np.asarray(W_hr), np.asarray(W_hz), np.asarray(W_hn)], axis=1
    ).astype(np_dt)
    xTb = np.ascontiguousarray(x_t.T).astype(np_dt)   # [IN, B]
    hTb = np.ascontiguousarray(h_t.T).astype(np_dt)   # [H, B]
    bcat = np.concatenate(
        [np.asarray(b_r), np.asarray(b_z), np.asarray(b_n)]
    ).astype(np.float32)
    use_bias = bool(np.any(bcat))

    in_maps = []
    for c in range(N_CORES):
        sl = slice(c * BL, (c + 1) * BL)
        m = {
            "xT": np.ascontiguousarray(xTb[:, sl]),
            "hT": np.ascontiguousarray(hTb[:, sl]),
            "hN": np.ascontiguousarray(h_t[sl]).astype(
                ml_dtypes.bfloat16 if HN_BF16 else np.float32
            ),
            "wi": Wi,
            "wh": Wh,
        }
        if use_bias:
            m["bias"] = np.tile(bcat[None, :], (P, 1))
        in_maps.append(m)
    return in_maps, use_bias


def kernel(x_t, h_t, W_ir, W_hr, b_r, W_iz, W_hz, b_z, W_in, W_hn, b_n):
    in_maps, use_bias = make_in_maps(
        x_t, h_t, W_ir, W_hr, b_r, W_iz, W_hz, b_z, W_in, W_hn, b_n
    )
    nc = build_program(use_bias=use_bias)
    res = run_bass_kernel_spmd(nc, in_maps, core_ids=list(range(N_CORES)))
    return np.concatenate(
        [res.results[c]["out"] for c in range(N_CORES)], axis=0
    ).astype(np.float32)



# revision 7
# speedup vs baseline: 1.1788x; 1.1788x over previous
"""GRU cell (nn.GRUCell) on 8 Trainium2 NeuronCores.

Strategy: data-parallel over the batch dim (16384 -> 2048 rows/core).
The 6 weight matrices are concatenated host-side into Wi=[IN,3H],
Wh=[H,3H] and replicated to every core; x_t/h_t are pre-transposed on
the host so each core can use batch-column slices of x^T/h^T directly
as the matmul stationary operand (lhsT, contraction dim on partitions)
without any on-device transpose.

Matmul dtype: float32r (fp32 bits, PE "replicated" mode) keeps near-fp32
accuracy; bf16 halves DMA traffic but amplifies error through the
r * gh_n product. Selected by `MM_DTYPE`.

Per core, per 128-row batch tile:
  r/z gates accumulate x- and h-side contributions in one PSUM group
  (8 matmuls each); the n gate needs gi_n and gh_n separately (the
  reset gate scales only the hidden contribution), 4 matmuls each.
  Gates are evaluated with ScalarE sigmoid/tanh straight out of PSUM,
  the blend (1-z)*n + z*h runs on VectorE in fp32, and the result is
  DMA'd out in natural [batch, H] layout.
"""

import numpy as np
import ml_dtypes

import concourse.mybir as mybir
from concourse import bacc
import concourse.tile as tile
from concourse.bass_utils import run_bass_kernel_spmd

N_CORES = 8
B, IN, H = 16384, 512, 512
BL = B // N_CORES          # batch rows per core
P = 128                    # SBUF partitions
MT = BL // P               # batch tiles per core
KC = IN // P               # contraction chunks per GEMM side
H3 = 3 * H
F32 = mybir.dt.float32
F32R = mybir.dt.float32r
BF16 = mybir.dt.bfloat16
SIG = mybir.ActivationFunctionType.Sigmoid
TANH = mybir.ActivationFunctionType.Tanh

MM_DTYPE = "bf16"          # "f32r" | "bf16"
HN_BF16 = "direct"         # False: f32 load; "dma": cast-on-DMA; "direct": bf16 tile into DVE
OUT_BF16 = False           # write out in bf16, upcast host-side


def build_program(
    use_bias=False,
    loop_n=1,
    enable_asserts=False,
    mm_dtype=None,
    unroll=1,
    resident_outside=False,
    wbufs=2,
    out_bf16=None,
):
    mm_dtype = mm_dtype or MM_DTYPE
    in_dt = BF16 if mm_dtype == "bf16" else F32R
    out_bf16 = OUT_BF16 if out_bf16 is None else out_bf16
    out_dt = BF16 if out_bf16 else F32

    def mm_ap(ap):
        return ap

    nc = bacc.Bacc(
        "TRN2",
        target_bir_lowering=False,
        debug=False,
        enable_asserts=enable_asserts,
        num_devices=N_CORES,
    )
    xT = nc.dram_tensor("xT", [IN, BL], in_dt, kind="ExternalInput").ap()
    hT = nc.dram_tensor("hT", [H, BL], in_dt, kind="ExternalInput").ap()
    hN = nc.dram_tensor(
        "hN", [BL, H], BF16 if HN_BF16 else F32, kind="ExternalInput"
    ).ap()
    wi = nc.dram_tensor("wi", [IN, H3], in_dt, kind="ExternalInput").ap()
    wh = nc.dram_tensor("wh", [H, H3], in_dt, kind="ExternalInput").ap()
    bias = (
        nc.dram_tensor("bias", [P, H3], F32, kind="ExternalInput").ap()
        if use_bias
        else None
    )
    out = nc.dram_tensor("out", [BL, H], out_dt, kind="ExternalOutput").ap()

    xT_c = xT.rearrange("(k p) b -> k p b", p=P)    # [KC, 128, BL]
    hT_c = hT.rearrange("(k p) b -> k p b", p=P)
    wi_c = wi.rearrange("(k p) n -> k p n", p=P)    # [KC, 128, 3H]
    wh_c = wh.rearrange("(k p) n -> k p n", p=P)
    hN_m = hN.rearrange("(m p) n -> m p n", p=P)    # [MT, 128, H]
    out_m = out.rearrange("(m p) n -> m p n", p=P)

    with tile.TileContext(nc) as tc:
        with (
            tc.tile_pool(name="resident", bufs=wbufs) as rpool,
            tc.tile_pool(name="stream", bufs=3) as spool,
            tc.tile_pool(name="psum", bufs=2, space="PSUM") as ppool,
        ):

            def load_resident():
                wi_sb, wh_sb = [], []
                for k in range(KC):
                    t = rpool.tile([P, H3], in_dt, tag=f"wi{k}")
                    nc.sync.dma_start(out=t, in_=wi_c[k])
                    wi_sb.append(t)
                for k in range(KC):
                    t = rpool.tile([P, H3], in_dt, tag=f"wh{k}")
                    nc.sync.dma_start(out=t, in_=wh_c[k])
                    wh_sb.append(t)
                b_sb = None
                if use_bias:
                    b_sb = rpool.tile([P, H3], F32, tag="bias")
                    nc.sync.dma_start(out=b_sb, in_=bias)
                return wi_sb, wh_sb, b_sb

            GS = 4               # m-tiles per activation slice group
            W_SL = GS * P        # slice width in batch columns

            def body(res):
                wi_sb, wh_sb, b_sb = res
                for m in range(MT):
                    mi = m % GS
                    if mi == 0:
                        g = m // GS
                        gsl = slice(g * W_SL, (g + 1) * W_SL)
                        xs, hs = [], []
                        for k in range(KC):
                            t = spool.tile([P, W_SL], in_dt, tag=f"xs{k}")
                            nc.sync.dma_start(out=t, in_=xT_c[k][:, gsl])
                            xs.append(t)
                        for k in range(KC):
                            t = spool.tile([P, W_SL], in_dt, tag=f"hs{k}")
                            nc.sync.dma_start(out=t, in_=hT_c[k][:, gsl])
                            hs.append(t)

                    if HN_BF16 == "direct":
                        h_sb = spool.tile([P, H], BF16, tag="h")
                        nc.sync.dma_start(out=h_sb, in_=hN_m[m])
                    elif HN_BF16 == "dma":
                        h_sb = spool.tile([P, H], F32, tag="h")
                        # bf16 -> f32 cast during DMA (SWDGE)
                        nc.gpsimd.dma_start(out=h_sb, in_=hN_m[m])
                    else:
                        h_sb = spool.tile([P, H], F32, tag="h")
                        nc.sync.dma_start(out=h_sb, in_=hN_m[m])

                    r_ps = ppool.tile([P, H], F32, tag="r")
                    z_ps = ppool.tile([P, H], F32, tag="z")
                    gin_ps = ppool.tile([P, H], F32, tag="gin")
                    ghn_ps = ppool.tile([P, H], F32, tag="ghn")

                    ms = slice(mi * P, (mi + 1) * P)
                    for k in range(KC):
                        lx = mm_ap(xs[k][:, ms])
                        w = wi_sb[k]
                        nc.tensor.matmul(
                            r_ps, lx, mm_ap(w[:, 0:H]), start=(k == 0), stop=False
                        )
                        nc.tensor.matmul(
                            z_ps, lx, mm_ap(w[:, H : 2 * H]), start=(k == 0), stop=False
                        )
                        nc.tensor.matmul(
                            gin_ps,
                            lx,
                            mm_ap(w[:, 2 * H : 3 * H]),
                            start=(k == 0),
                            stop=(k == KC - 1),
                        )
                    for k in range(KC):
                        lh = mm_ap(hs[k][:, ms])
                        w = wh_sb[k]
                        nc.tensor.matmul(
                            r_ps, lh, mm_ap(w[:, 0:H]), start=False, stop=(k == KC - 1)
                        )
                        nc.tensor.matmul(
                            z_ps,
                            lh,
                            mm_ap(w[:, H : 2 * H]),
                            start=False,
                            stop=(k == KC - 1),
                        )
                        nc.tensor.matmul(
                            ghn_ps,
                            lh,
                            mm_ap(w[:, 2 * H : 3 * H]),
                            start=(k == 0),
                            stop=(k == KC - 1),
                        )

                    if use_bias:
                        nc.vector.tensor_add(r_ps, r_ps, b_sb[:, 0:H])
                        nc.vector.tensor_add(z_ps, z_ps, b_sb[:, H : 2 * H])
                        nc.vector.tensor_add(gin_ps, gin_ps, b_sb[:, 2 * H : 3 * H])

                    r_sb = spool.tile([P, H], F32, tag="r_sb")
                    nc.scalar.activation(r_sb, r_ps, SIG)
                    z_sb = spool.tile([P, H], F32, tag="z_sb")
                    nc.scalar.activation(z_sb, z_ps, SIG)

                    t_sb = spool.tile([P, H], F32, tag="t_sb")
                    nc.vector.tensor_mul(t_sb, r_sb, ghn_ps)
                    np_sb = spool.tile([P, H], F32, tag="np_sb")
                    nc.vector.tensor_add(np_sb, t_sb, gin_ps)
                    n_sb = spool.tile([P, H], F32, tag="n_sb")
                    nc.scalar.activation(n_sb, np_sb, TANH)

                    # out = n + z * (h - n)  ==  (1-z)*n + z*h
                    d_sb = spool.tile([P, H], F32, tag="d_sb")
                    nc.vector.tensor_sub(d_sb, h_sb, n_sb)
                    e_sb = spool.tile([P, H], F32, tag="e_sb")
                    nc.vector.tensor_mul(e_sb, z_sb, d_sb)
                    o_sb = spool.tile([P, H], out_dt, tag="o_sb")
                    nc.vector.tensor_add(o_sb, n_sb, e_sb)
                    nc.sync.dma_start(out=out_m[m], in_=o_sb)

            if loop_n == 1:
                res = load_resident()
                for _ in range(unroll):
                    body(res)
            elif resident_outside:
                res = load_resident()
                with tc.For_i(0, loop_n, 1):
                    for _ in range(unroll):
                        body(res)
            else:
                with tc.For_i(0, loop_n, 1):
                    for _ in range(unroll):
                        res = load_resident()
                        body(res)

    nc.compile()
    return nc


def make_in_maps(
    x_t, h_t, W_ir, W_hr, b_r, W_iz, W_hz, b_z, W_in, W_hn, b_n, mm_dtype=None
):
    mm_dtype = mm_dtype or MM_DTYPE
    np_dt = ml_dtypes.bfloat16 if mm_dtype == "bf16" else np.float32
    x_t = np.asarray(x_t, dtype=np.float32)
    h_t = np.asarray(h_t, dtype=np.float32)
    Wi = np.concatenate(
        [np.asarray(W_ir), np.asarray(W_iz), np.asarray(W_in)], axis=1
    ).astype(np_dt)
    Wh = np.concatenate(
        [np.asarray(W_hr), np.asarray(W_hz), np.asarray(W_hn)], axis=1
    ).astype(np_dt)
    xTb = np.ascontiguousarray(x_t.T).astype(np_dt)   # [IN, B]
    hTb = np.ascontiguousarray(h_t.T).astype(np_dt)   # [H, B]
    bcat = np.concatenate(
        [np.asarray(b_r), np.asarray(b_z), np.asarray(b_n)]
    ).astype(np.float32)
    use_bias = bool(np.any(bcat))

    in_maps = []
    for c in range(N_CORES):
        sl = slice(c * BL, (c + 1) * BL)
        m = {
            "xT": np.ascontiguousarray(xTb[:, sl]),
            "hT": np.ascontiguousarray(hTb[:, sl]),
            "hN": np.ascontiguousarray(h_t[sl]).astype(
                ml_dtypes.bfloat16 if HN_BF16 else np.float32
            ),
            "wi": Wi,
            "wh": Wh,
        }
        if use_bias:
            m["bias"] = np.tile(bcat[None, :], (P, 1))
        in_maps.append(m)
    return in_maps, use_bias


def kernel(x_t, h_t, W_ir, W_hr, b_r, W_iz, W_hz, b_z, W_in, W_hn, b_n):
    in_maps, use_bias = make_in_maps(
        x_t, h_t, W_ir, W_hr, b_r, W_iz, W_hz, b_z, W_in, W_hn, b_n
    )
    nc = build_program(use_bias=use_bias)
    res = run_bass_kernel_spmd(nc, in_maps, core_ids=list(range(N_CORES)))
    return np.concatenate(
        [np.asarray(res.results[c]["out"]) for c in range(N_CORES)], axis=0
    ).astype(np.float32)



# revision 12
# speedup vs baseline: 2.9275x; 2.4834x over previous
"""GRU cell (nn.GRUCell) on 8 Trainium2 NeuronCores.

Strategy: data-parallel over the batch dim (16384 -> 2048 rows/core).
The 6 weight matrices are concatenated host-side into Wi=[IN,3H],
Wh=[H,3H] and replicated to every core; x_t/h_t are pre-transposed on
the host so each core can use batch-column slices of x^T/h^T directly
as the matmul stationary operand (lhsT, contraction dim on partitions)
without any on-device transpose.

Matmul dtype: float32r (fp32 bits, PE "replicated" mode) keeps near-fp32
accuracy; bf16 halves DMA traffic but amplifies error through the
r * gh_n product. Selected by `MM_DTYPE`.

Per core, per 128-row batch tile:
  r/z gates accumulate x- and h-side contributions in one PSUM group
  (8 matmuls each); the n gate needs gi_n and gh_n separately (the
  reset gate scales only the hidden contribution), 4 matmuls each.
  Gates are evaluated with ScalarE sigmoid/tanh straight out of PSUM,
  the blend (1-z)*n + z*h runs on VectorE in fp32, and the result is
  DMA'd out in natural [batch, H] layout.
"""

import numpy as np
import ml_dtypes

import concourse.mybir as mybir
from concourse import bacc
import concourse.tile as tile
from concourse.bass_utils import run_bass_kernel_spmd

N_CORES = 8
B, IN, H = 16384, 512, 512
BL = B // N_CORES          # batch rows per core
P = 128                    # SBUF partitions
MT = BL // P               # batch tiles per core
KC = IN // P               # contraction chunks per GEMM side
H3 = 3 * H
F32 = mybir.dt.float32
F32R = mybir.dt.float32r
BF16 = mybir.dt.bfloat16
SIG = mybir.ActivationFunctionType.Sigmoid
TANH = mybir.ActivationFunctionType.Tanh

MM_DTYPE = "bf16"          # "f32r" | "bf16"
HN_BF16 = "direct"         # False: f32 load; "dma": cast-on-DMA; "direct": bf16 tile into DVE
OUT_BF16 = False           # write out in bf16, upcast host-side


def build_program(
    use_bias=False,
    loop_n=1,
    enable_asserts=False,
    mm_dtype=None,
    unroll=1,
    resident_outside=False,
    wbufs=2,
    out_bf16=None,
    qsplit=True,
):
    mm_dtype = mm_dtype or MM_DTYPE
    in_dt = BF16 if mm_dtype == "bf16" else F32R
    out_bf16 = OUT_BF16 if out_bf16 is None else out_bf16
    out_dt = BF16 if out_bf16 else F32

    def mm_ap(ap):
        return ap

    nc = bacc.Bacc(
        "TRN2",
        target_bir_lowering=False,
        debug=False,
        enable_asserts=enable_asserts,
        num_devices=N_CORES,
    )
    xT = nc.dram_tensor("xT", [IN, BL], in_dt, kind="ExternalInput").ap()
    hT = nc.dram_tensor("hT", [H, BL], in_dt, kind="ExternalInput").ap()
    hN = nc.dram_tensor(
        "hN", [BL, H], BF16 if HN_BF16 else F32, kind="ExternalInput"
    ).ap()
    wi = nc.dram_tensor("wi", [IN, H3], in_dt, kind="ExternalInput").ap()
    wh = nc.dram_tensor("wh", [H, H3], in_dt, kind="ExternalInput").ap()
    bias = (
        nc.dram_tensor("bias", [P, H3], F32, kind="ExternalInput").ap()
        if use_bias
        else None
    )
    out = nc.dram_tensor("out", [BL, H], out_dt, kind="ExternalOutput").ap()

    xT_c = xT.rearrange("(k p) b -> k p b", p=P)    # [KC, 128, BL]
    hT_c = hT.rearrange("(k p) b -> k p b", p=P)
    wi_c = wi.rearrange("(k p) n -> k p n", p=P)    # [KC, 128, 3H]
    wh_c = wh.rearrange("(k p) n -> k p n", p=P)
    hN_m = hN.rearrange("(m p) n -> m p n", p=P)    # [MT, 128, H]
    out_m = out.rearrange("(m p) n -> m p n", p=P)

    with tile.TileContext(nc) as tc:
        with (
            tc.tile_pool(name="resident", bufs=wbufs) as rpool,
            tc.tile_pool(name="stream", bufs=3) as spool,
            tc.tile_pool(name="psum", bufs=2, space="PSUM") as ppool,
        ):

            # DMA queue split: xs/hs group loads stay on the SP HWDGE
            # queue; weight + out-store traffic goes to the idle gpsimd
            # SWDGE queue; hN loads to the Activation HWDGE queue. One
            # queue serializes all transfers; three run in parallel.
            qsplit = qsplit if isinstance(qsplit, str) else (
                "full" if qsplit else "none"
            )
            weng = nc.gpsimd if qsplit in ("full", "pool") else nc.sync
            heng = nc.scalar if qsplit in ("full", "act") else nc.sync
            oeng = nc.gpsimd if qsplit in ("full", "pool") else nc.sync

            def load_resident():
                wi_sb, wh_sb = [], []
                for k in range(KC):
                    t = rpool.tile([P, H3], in_dt, tag=f"wi{k}")
                    weng.dma_start(out=t, in_=wi_c[k])
                    wi_sb.append(t)
                for k in range(KC):
                    t = rpool.tile([P, H3], in_dt, tag=f"wh{k}")
                    weng.dma_start(out=t, in_=wh_c[k])
                    wh_sb.append(t)
                b_sb = None
                if use_bias:
                    b_sb = rpool.tile([P, H3], F32, tag="bias")
                    nc.sync.dma_start(out=b_sb, in_=bias)
                return wi_sb, wh_sb, b_sb

            GS = 4               # m-tiles per activation slice group
            W_SL = GS * P        # slice width in batch columns

            def body(res):
                wi_sb, wh_sb, b_sb = res
                for m in range(MT):
                    mi = m % GS
                    if mi == 0:
                        g = m // GS
                        gsl = slice(g * W_SL, (g + 1) * W_SL)
                        xs, hs = [], []
                        for k in range(KC):
                            t = spool.tile([P, W_SL], in_dt, tag=f"xs{k}")
                            nc.sync.dma_start(out=t, in_=xT_c[k][:, gsl])
                            xs.append(t)
                        for k in range(KC):
                            t = spool.tile([P, W_SL], in_dt, tag=f"hs{k}")
                            nc.sync.dma_start(out=t, in_=hT_c[k][:, gsl])
                            hs.append(t)

                    if HN_BF16 == "direct":
                        h_sb = spool.tile([P, H], BF16, tag="h")
                        heng.dma_start(out=h_sb, in_=hN_m[m])
                    elif HN_BF16 == "dma":
                        h_sb = spool.tile([P, H], F32, tag="h")
                        # bf16 -> f32 cast during DMA (SWDGE)
                        nc.gpsimd.dma_start(out=h_sb, in_=hN_m[m])
                    else:
                        h_sb = spool.tile([P, H], F32, tag="h")
                        nc.sync.dma_start(out=h_sb, in_=hN_m[m])

                    r_ps = ppool.tile([P, H], F32, tag="r")
                    z_ps = ppool.tile([P, H], F32, tag="z")
                    gin_ps = ppool.tile([P, H], F32, tag="gin")
                    ghn_ps = ppool.tile([P, H], F32, tag="ghn")

                    ms = slice(mi * P, (mi + 1) * P)
                    for k in range(KC):
                        lx = mm_ap(xs[k][:, ms])
                        w = wi_sb[k]
                        nc.tensor.matmul(
                            r_ps, lx, mm_ap(w[:, 0:H]), start=(k == 0), stop=False
                        )
                        nc.tensor.matmul(
                            z_ps, lx, mm_ap(w[:, H : 2 * H]), start=(k == 0), stop=False
                        )
                        nc.tensor.matmul(
                            gin_ps,
                            lx,
                            mm_ap(w[:, 2 * H : 3 * H]),
                            start=(k == 0),
                            stop=(k == KC - 1),
                        )
                    for k in range(KC):
                        lh = mm_ap(hs[k][:, ms])
                        w = wh_sb[k]
                        nc.tensor.matmul(
                            r_ps, lh, mm_ap(w[:, 0:H]), start=False, stop=(k == KC - 1)
                        )
                        nc.tensor.matmul(
                            z_ps,
                            lh,
                            mm_ap(w[:, H : 2 * H]),
                            start=False,
                            stop=(k == KC - 1),
                        )
                        nc.tensor.matmul(
                            ghn_ps,
                            lh,
                            mm_ap(w[:, 2 * H : 3 * H]),
                            start=(k == 0),
                            stop=(k == KC - 1),
                        )

                    if use_bias:
                        nc.vector.tensor_add(r_ps, r_ps, b_sb[:, 0:H])
                        nc.vector.tensor_add(z_ps, z_ps, b_sb[:, H : 2 * H])
                        nc.vector.tensor_add(gin_ps, gin_ps, b_sb[:, 2 * H : 3 * H])

                    r_sb = spool.tile([P, H], F32, tag="r_sb")
                    nc.scalar.activation(r_sb, r_ps, SIG)
                    z_sb = spool.tile([P, H], F32, tag="z_sb")
                    nc.scalar.activation(z_sb, z_ps, SIG)

                    t_sb = spool.tile([P, H], F32, tag="t_sb")
                    nc.vector.tensor_mul(t_sb, r_sb, ghn_ps)
                    np_sb = spool.tile([P, H], F32, tag="np_sb")
                    nc.vector.tensor_add(np_sb, t_sb, gin_ps)
                    n_sb = spool.tile([P, H], F32, tag="n_sb")
                    nc.scalar.activation(n_sb, np_sb, TANH)

                    # out = n + z * (h - n)  ==  (1-z)*n + z*h
                    d_sb = spool.tile([P, H], F32, tag="d_sb")
                    nc.vector.tensor_sub(d_sb, h_sb, n_sb)
                    e_sb = spool.tile([P, H], F32, tag="e_sb")
                    nc.vector.tensor_mul(e_sb, z_sb, d_sb)
                    o_sb = spool.tile([P, H], out_dt, tag="o_sb")
                    nc.vector.tensor_add(o_sb, n_sb, e_sb)
                    oeng.dma_start(out=out_m[m], in_=o_sb)

            if loop_n == 1:
                res = load_resident()
                for _ in range(unroll):
                    body(res)
            elif resident_outside:
                res = load_resident()
                with tc.For_i(0, loop_n, 1):
                    for _ in range(unroll):
                        body(res)
            else:
                with tc.For_i(0, loop_n, 1):
                    for _ in range(unroll):
                        res = load_resident()
                        body(res)

    nc.compile()
    return nc


def make_in_maps(
    x_t, h_t, W_ir, W_hr, b_r, W_iz, W_hz, b_z, W_in, W_hn, b_n, mm_dtype=None
):
    mm_dtype = mm_dtype or MM_DTYPE
    np_dt = ml_dtypes.bfloat16 if mm_dtype == "bf16" else np.float32
    x_t = np.asarray(x_t, dtype=np.float32)
    h_t = np.asarray(h_t, dtype=np.float32)
    Wi = np.concatenate(
        [np.asarray(W_ir), np.asarray(W_iz), np.asarray(W_in)], axis=1
    ).astype(np_dt)
    Wh = np.concatenate(
        [np.asarray(W_hr), np.asarray(W_hz), np.asarray(W_hn)], axis=1
    ).astype(np_dt)
    xTb = np.ascontiguousarray(x_t.T).astype(np_dt)   # [IN, B]
    hTb = np.ascontiguousarray(h_t.T).astype(np_dt)   # [H, B]
    bcat = np.concatenate(
        [np.asarray(b_r), np.asarray(b_z), np.asarray(b_n)]
    ).astype(np.float32)
    use_bias = bool(np.any(bcat))

    in_maps = []
    for c in range(N_CORES):
        sl = slice(c * BL, (c + 1) * BL)
        m = {
            "xT": np.ascontiguousarray(xTb[:, sl]),
            "hT": np.ascontiguousarray(hTb[:, sl]),
            "hN": np.ascontiguousarray(h_t[sl]).astype(
                ml_dtypes.bfloat16 if HN_BF16 else np.float32
            ),
            "wi": Wi,
            "wh": Wh,
        }
        if use_bias:
            m["bias"] = np.tile(bcat[None, :], (P, 1))
        in_maps.append(m)
    return in_maps, use_bias


def kernel(x_t, h_t, W_ir, W_hr, b_r, W_iz, W_hz, b_z, W_in, W_hn, b_n):
    in_maps, use_bias = make_in_maps(
        x_t, h_t, W_ir, W_hr, b_r, W_iz, W_hz, b_z, W_in, W_hn, b_n
    )
    nc = build_program(use_bias=use_bias)
    res = run_bass_kernel_spmd(nc, in_maps, core_ids=list(range(N_CORES)))
    return np.concatenate(
        [np.asarray(res.results[c]["out"]) for c in range(N_CORES)], axis=0
    ).astype(np.float32)

